# revision 1
# baseline (speedup 1.0000x reference)
"""Trainium2 Bass kernel for DistributedAFNO2D.

Problem: x(2,768,256,256) f32; per-block (8 blocks of 96 ch) spectral MLP:
  out = irfft2( softshrink( W2*relu(W1*rfft2(x) + b1) + b2 ) ) + x
Block-diagonal channel mixing with shared-per-(u,v) complex 96x96 weights.

Sharding: block k -> core k (8 cores). No collectives. Each core handles
(2, 96, 256, 256) with its own block weights.

All FFTs are dense matmuls with probed DFT matrices (bf16 inputs, fp32 PSUM).
Dataflow per core, per batch b:
  Phase A (per channel c):
    S1 contract h: psY[w_chunk, (Yr-u256 | Yi-u256)] = x[h,w].T-matmul CHpack
    S2 contract w: psZ[u_chunk, (Zr-v129 | Zi-v129)] via R1/R2 consts
    -> Zbuf[b, u, c, 258] bf16 in DRAM
  Phase B (per pair of u):
    Z1 tile [98, 2, 258] (rows 96/97 = bias ones-pattern)
    mix1 (3 matmuls: W1R_aug*Z1 + split-sign W1I on swapped halves) + b1 (K-aug)
    relu (ACT) -> o1P [98, 2, 258] (rows 96/97 ones-pattern)
    mix2 (3 matmuls) + b2 -> psum t
    softshrink: c=clamp(t,+-lam); s=t-c (DVE) -> Sbuf[b, c, u, 258] bf16
  Phase C (per channel c):
    Q^T (v 1..128 on partitions) = s-slices x CHIpack matmuls; combines (DVE)
    DC term q0 = (CHIr.sr0 - CHIi.si0)/16 (N=1 matmuls)
    out[h,w] = QrT.Gc + QiT.Gs (matmuls) + x + q0  (DVE stt)
"""
import os
import sys
import numpy as np

sys.path.insert(0, "/opt/trn_rl_repo")

import ml_dtypes

BF16 = ml_dtypes.bfloat16

H = 256
W = 256
NV = W // 2 + 1  # 129
BLK = 96
NCORES = 8
B = 2
LAM = 0.01


def make_host_consts():
    """All packed constant matrices (numpy bf16) via probing np.fft."""
    I = np.eye(H, dtype=np.float64)
    F = np.fft.fft(I, axis=0, norm='ortho')       # F[u,h]; F@x = fft(x)
    Fi = np.fft.ifft(I, axis=0, norm='ortho')     # Fi[h,u]
    CHr = F.real.T.copy()                          # [h,u]
    CHi = F.imag.T.copy()
    EWr = F.real.T[:, :NV].copy()                  # [w,v]
    EWi = F.imag.T[:, :NV].copy()
    CHIr = Fi.real.T.copy()                        # [u,h]
    CHIi = Fi.imag.T.copy()
    Ir = np.eye(NV)
    Gc = np.fft.irfft(Ir, n=W, axis=-1, norm='ortho')        # [v,w]
    Gs = np.fft.irfft(1j * Ir, n=W, axis=-1, norm='ortho')   # [v,w]

    c = {}
    # CHpack [2][128, 512]: rows h (chunk), cols [CHr-u | CHi-u]
    c['chpack'] = np.stack([
        np.concatenate([CHr[j * 128:(j + 1) * 128, :], CHi[j * 128:(j + 1) * 128, :]], axis=1)
        for j in range(2)])
    # R1 [2][128, 258] = [EWr | EWi]; R2 = [-EWi | EWr] rows w chunk
    c['r1'] = np.stack([
        np.concatenate([EWr[j * 128:(j + 1) * 128], EWi[j * 128:(j + 1) * 128]], axis=1)
        for j in range(2)])
    c['r2'] = np.stack([
        np.concatenate([-EWi[j * 128:(j + 1) * 128], EWr[j * 128:(j + 1) * 128]], axis=1)
        for j in range(2)])
    # CHIpack [2][128, 512]: rows u chunk, cols [CHIr-h | CHIi-h]
    c['chipack'] = np.stack([
        np.concatenate([CHIr[j * 128:(j + 1) * 128], CHIi[j * 128:(j + 1) * 128]], axis=1)
        for j in range(2)])
    # NCHI [2][128, 256] = -CHIi rows u chunk
    c['nchi'] = np.stack([-CHIi[j * 128:(j + 1) * 128] for j in range(2)])
    # G tiles rows v=1..128
    c['gc'] = Gc[1:129]
    c['gs'] = Gs[1:129]
    return {k: v.astype(BF16) for k, v in c.items()}


def make_weight_consts(w1k, b1k, w2k, b2k):
    """Augmented weight matrices for one block.
    w1k/w2k: (96, 96, 2) [i, o, ri]; b1k/b2k: (96, 2) [o, ri]."""
    return {
        'w1r': w1k[..., 0].astype(BF16),
        'w1i': w1k[..., 1].astype(BF16),
        'w1in': (-w1k[..., 1]).astype(BF16),
        'w2r': w2k[..., 0].astype(BF16),
        'w2i': w2k[..., 1].astype(BF16),
        'w2in': (-w2k[..., 1]).astype(BF16),
    }


def build_nc():
    import concourse.bass as bass
    import concourse.tile as tile
    from concourse import bacc, mybir

    dt = mybir.dt
    nc = bacc.Bacc("TRN2", target_bir_lowering=False, debug=False)

    # I/O
    x32 = nc.dram_tensor("x32", [B, BLK, H, W], dt.float32, kind="ExternalInput").ap()
    xbf = nc.dram_tensor("xbf", [B, BLK, H, W], dt.bfloat16, kind="ExternalInput").ap()
    chpack = nc.dram_tensor("chpack", [2, 128, 512], dt.bfloat16, kind="ExternalInput").ap()
    r1 = nc.dram_tensor("r1", [2, 128, 258], dt.bfloat16, kind="ExternalInput").ap()
    r2 = nc.dram_tensor("r2", [2, 128, 258], dt.bfloat16, kind="ExternalInput").ap()
    chipack = nc.dram_tensor("chipack", [2, 128, 512], dt.bfloat16, kind="ExternalInput").ap()
    nchi = nc.dram_tensor("nchi", [2, 128, 256], dt.bfloat16, kind="ExternalInput").ap()
    gc = nc.dram_tensor("gc", [128, 256], dt.bfloat16, kind="ExternalInput").ap()
    gs = nc.dram_tensor("gs", [128, 256], dt.bfloat16, kind="ExternalInput").ap()
    wts = {n: nc.dram_tensor(n, [96, 96], dt.bfloat16, kind="ExternalInput").ap()
           for n in ['w1r', 'w1i', 'w1in', 'w2r', 'w2i', 'w2in']}
    b1cols = nc.dram_tensor("b1cols", [96, 2], dt.float32, kind="ExternalInput").ap()
    b2cols = nc.dram_tensor("b2cols", [96, 4], dt.float32, kind="ExternalInput").ap()
    out = nc.dram_tensor("out", [B, BLK, H, W], dt.float32, kind="ExternalOutput").ap()

    # DRAM scratch
    zbuf = nc.dram_tensor("zbuf", [B, H, BLK, 258], dt.bfloat16).ap()
    sbuf_d = nc.dram_tensor("sbufd", [B, BLK, H, 258], dt.bfloat16).ap()


    with tile.TileContext(nc) as tc:
        from contextlib import ExitStack
        with ExitStack() as ctx:
            consts = ctx.enter_context(tc.tile_pool(name="consts", bufs=1))
            pa_x = ctx.enter_context(tc.tile_pool(name="pa_x", bufs=4))
            pa_y = ctx.enter_context(tc.tile_pool(name="pa_y", bufs=4))
            pa_z = ctx.enter_context(tc.tile_pool(name="pa_z", bufs=4))
            pb_s = ctx.enter_context(tc.tile_pool(name="pb_s", bufs=4))
            pc_in = ctx.enter_context(tc.tile_pool(name="pc_in", bufs=4))
            pc_q = ctx.enter_context(tc.tile_pool(name="pc_q", bufs=4))
            pc_o = ctx.enter_context(tc.tile_pool(name="pc_o", bufs=4))
            # Single PSUM pool: 3 shared tags x (3+3+2) bufs x 1 bank = 8 banks
            psum = ctx.enter_context(tc.tile_pool(name="psum", bufs=1, space="PSUM"))

            # ---- Load constants (one [128, X] tile per chunk) ----
            def chunked_const(name, ap_, ncols):
                ts = []
                for j in range(2):
                    t = consts.tile([128, ncols], dt.bfloat16, tag=f"{name}{j}", name=f"{name}{j}")
                    nc.sync.dma_start(out=t, in_=ap_[j])
                    ts.append(t)
                return ts

            t_ch = chunked_const("t_ch", chpack, 512)
            t_r1 = chunked_const("t_r1", r1, 258)
            t_r2 = chunked_const("t_r2", r2, 258)
            t_chi = chunked_const("t_chi", chipack, 512)
            t_nchi = chunked_const("t_nchi", nchi, 256)
            t_gc = consts.tile([128, 256], dt.bfloat16, tag="t_gc", name="t_gc")
            nc.sync.dma_start(out=t_gc, in_=gc)
            t_gs = consts.tile([128, 256], dt.bfloat16, tag="t_gs", name="t_gs")
            nc.sync.dma_start(out=t_gs, in_=gs)
            t_w = {}
            for n, ap_ in wts.items():
                t_w[n] = consts.tile([96, 96], dt.bfloat16, tag=f"t_{n}", name=f"t_{n}")
                nc.sync.dma_start(out=t_w[n], in_=ap_)

            t_b2 = consts.tile([96, 4], dt.float32, tag="t_b2", name="t_b2")
            nc.sync.dma_start(out=t_b2, in_=b2cols)
            t_b1 = consts.tile([96, 2], dt.float32, tag="t_b1", name="t_b1")
            nc.sync.dma_start(out=t_b1, in_=b1cols)

            for b in range(B):
                # ================= Phase A =================
                for c in range(BLK):
                    xt0 = pa_x.tile([128, 256], dt.bfloat16, tag="xt0", name="xt0")
                    nc.sync.dma_start(out=xt0, in_=xbf[b, c, 0:128, :])
                    xt1 = pa_x.tile([128, 256], dt.bfloat16, tag="xt1", name="xt1")
                    nc.sync.dma_start(out=xt1, in_=xbf[b, c, 128:256, :])

                    ys = []
                    for wc in range(2):
                        psy = psum.tile([128, 512], dt.float32, tag="psA", name="psy", bufs=3)
                        nc.tensor.matmul(psy, lhsT=xt0[:, wc * 128:(wc + 1) * 128],
                                         rhs=t_ch[0], start=True, stop=False)
                        nc.tensor.matmul(psy, lhsT=xt1[:, wc * 128:(wc + 1) * 128],
                                         rhs=t_ch[1], start=False, stop=True)
                        y = pa_y.tile([128, 512], dt.bfloat16, tag=f"y{wc}", name=f"y{wc}")
                        nc.scalar.copy(y, psy)
                        ys.append(y)

                    for uc in range(2):
                        psz = psum.tile([128, 512], dt.float32, tag="psB", name="psz", bufs=3)
                        us = slice(uc * 128, (uc + 1) * 128)
                        us2 = slice(256 + uc * 128, 256 + (uc + 1) * 128)
                        nc.tensor.matmul(psz[:, 0:258], lhsT=ys[0][:, us], rhs=t_r1[0], start=True, stop=False)
                        nc.tensor.matmul(psz[:, 0:258], lhsT=ys[0][:, us2], rhs=t_r2[0], start=False, stop=False)
                        nc.tensor.matmul(psz[:, 0:258], lhsT=ys[1][:, us], rhs=t_r1[1], start=False, stop=False)
                        nc.tensor.matmul(psz[:, 0:258], lhsT=ys[1][:, us2], rhs=t_r2[1], start=False, stop=True)
                        zt = pa_z.tile([128, 258], dt.bfloat16, tag="zt", name="zt")
                        nc.scalar.copy(zt, psz[:, 0:258])
                        nc.sync.dma_start(out=zbuf[b, uc * 128:(uc + 1) * 128, c, :], in_=zt)

                # ================= Phase B =================
                for u in range(H):
                    z1 = pb_s.tile([96, 258], dt.bfloat16, tag="z1", name="z1")
                    nc.gpsimd.dma_start(out=z1, in_=zbuf[b, u, :, :])

                    ps1 = psum.tile([96, 512], dt.float32, tag="psA", name="ps1", bufs=3)
                    nc.tensor.matmul(ps1[:, 0:258], lhsT=t_w['w1r'], rhs=z1, start=True, stop=False,
                                     skip_group_check=True)
                    nc.tensor.matmul(ps1[:, 0:129], lhsT=t_w['w1in'], rhs=z1[:, 129:258],
                                     start=False, stop=True, skip_group_check=True)
                    nc.tensor.matmul(ps1[:, 129:258], lhsT=t_w['w1i'], rhs=z1[:, 0:129],
                                     start=False, stop=True, skip_group_check=True)

                    o1 = pb_s.tile([96, 258], dt.bfloat16, tag="o1", name="o1")
                    nc.scalar.activation(o1[:, 0:129], ps1[:, 0:129],
                                         mybir.ActivationFunctionType.Relu, bias=t_b1[:, 0:1])
                    nc.scalar.activation(o1[:, 129:258], ps1[:, 129:258],
                                         mybir.ActivationFunctionType.Relu, bias=t_b1[:, 1:2])

                    ps2 = psum.tile([96, 512], dt.float32, tag="psB", name="ps2", bufs=3)
                    nc.tensor.matmul(ps2[:, 0:258], lhsT=t_w['w2r'], rhs=o1, start=True, stop=False,
                                     skip_group_check=True)
                    nc.tensor.matmul(ps2[:, 0:129], lhsT=t_w['w2in'], rhs=o1[:, 129:258],
                                     start=False, stop=True, skip_group_check=True)
                    nc.tensor.matmul(ps2[:, 129:258], lhsT=t_w['w2i'], rhs=o1[:, 0:129],
                                     start=False, stop=True, skip_group_check=True)

                    # softshrink with b2 folded into clamp bounds:
                    # s = o2 - clamp(o2, -lam-b2, lam-b2)
                    cl = pb_s.tile([96, 258], dt.float32, tag="cl", name="cl")
                    nc.vector.tensor_scalar(cl[:, 0:129], ps2[:, 0:129], t_b2[:, 0:1], t_b2[:, 1:2],
                                            mybir.AluOpType.min, mybir.AluOpType.max)
                    nc.vector.tensor_scalar(cl[:, 129:258], ps2[:, 129:258], t_b2[:, 2:3], t_b2[:, 3:4],
                                            mybir.AluOpType.min, mybir.AluOpType.max)
                    st = pb_s.tile([96, 258], dt.bfloat16, tag="st", name="st")
                    nc.vector.tensor_tensor(st, ps2[:, 0:258], cl, mybir.AluOpType.subtract)
                    nc.sync.dma_start(out=sbuf_d[b, :, u, :], in_=st)

                # ================= Phase C =================
                for c in range(BLK):
                    st0 = pc_in.tile([128, 258], dt.bfloat16, tag="st0", name="st0")
                    nc.gpsimd.dma_start(out=st0, in_=sbuf_d[b, c, 0:128, :])
                    st1 = pc_in.tile([128, 258], dt.bfloat16, tag="st1", name="st1")
                    nc.gpsimd.dma_start(out=st1, in_=sbuf_d[b, c, 128:256, :])

                    # QrT = sr.CHIr - si.CHIi ; QiT = sr.CHIi + si.CHIr  (psum accum)
                    psa = psum.tile([128, 256], dt.float32, tag="psA", name="psa", bufs=3)
                    nc.tensor.matmul(psa, lhsT=st0[:, 1:129], rhs=t_chi[0][:, 0:256], start=True, stop=False)
                    nc.tensor.matmul(psa, lhsT=st1[:, 1:129], rhs=t_chi[1][:, 0:256], start=False, stop=False)
                    nc.tensor.matmul(psa, lhsT=st0[:, 130:258], rhs=t_nchi[0], start=False, stop=False)
                    nc.tensor.matmul(psa, lhsT=st1[:, 130:258], rhs=t_nchi[1], start=False, stop=True)
                    psb = psum.tile([128, 256], dt.float32, tag="psB", name="psb", bufs=3)
                    nc.tensor.matmul(psb, lhsT=st0[:, 1:129], rhs=t_chi[0][:, 256:512], start=True, stop=False)
                    nc.tensor.matmul(psb, lhsT=st1[:, 1:129], rhs=t_chi[1][:, 256:512], start=False, stop=False)
                    nc.tensor.matmul(psb, lhsT=st0[:, 130:258], rhs=t_chi[0][:, 0:256], start=False, stop=False)
                    nc.tensor.matmul(psb, lhsT=st1[:, 130:258], rhs=t_chi[1][:, 0:256], start=False, stop=True)

                    qr = pc_q.tile([128, 256], dt.bfloat16, tag="qr", name="qr")
                    nc.scalar.copy(qr, psa)
                    qi = pc_q.tile([128, 256], dt.bfloat16, tag="qi", name="qi")
                    nc.scalar.copy(qi, psb)

                    # DC (v=0) term -> q0 per h-chunk
                    psq = psum.tile([128, 2], dt.float32, tag="psC", name="psq", bufs=2)
                    for hc in range(2):
                        hs = slice(hc * 128, (hc + 1) * 128)
                        nc.tensor.matmul(psq[:, hc:hc + 1], lhsT=t_chi[0][:, hs], rhs=st0[:, 0:1],
                                         start=(hc == 0), stop=False, skip_group_check=True)
                        nc.tensor.matmul(psq[:, hc:hc + 1], lhsT=t_nchi[0][:, hs], rhs=st0[:, 129:130],
                                         start=False, stop=False, skip_group_check=True)
                        nc.tensor.matmul(psq[:, hc:hc + 1], lhsT=t_chi[1][:, hs], rhs=st1[:, 0:1],
                                         start=False, stop=False, skip_group_check=True)
                        nc.tensor.matmul(psq[:, hc:hc + 1], lhsT=t_nchi[1][:, hs], rhs=st1[:, 129:130],
                                         start=False, stop=True, skip_group_check=True)
                    q0 = pc_q.tile([128, 2], dt.float32, tag="q0", name="q0")
                    nc.vector.tensor_scalar_mul(q0, psq, 1.0 / 16.0)

                    for hc in range(2):
                        hs = slice(hc * 128, (hc + 1) * 128)
                        pso = psum.tile([128, 512], dt.float32, tag="psC", name="pso", bufs=2)[:, 0:256]
                        nc.tensor.matmul(pso, lhsT=qr[:, hs], rhs=t_gc, start=True, stop=False)
                        nc.tensor.matmul(pso, lhsT=qi[:, hs], rhs=t_gs, start=False, stop=True)
                        xt = pc_o.tile([128, 256], dt.float32, tag="xt", name="xt")
                        nc.sync.dma_start(out=xt, in_=x32[b, c, hs, :])
                        ot = pc_o.tile([128, 256], dt.float32, tag="ot", name="ot")
                        nc.vector.scalar_tensor_tensor(
                            ot, xt, q0[:, hc:hc + 1], pso,
                            mybir.AluOpType.add, mybir.AluOpType.add)
                        nc.sync.dma_start(out=out[b, c, hs, :], in_=ot)
    nc.compile()
    return nc


_NC_CACHE = {}


def _get_nc():
    if 'nc' not in _NC_CACHE:
        _NC_CACHE['nc'] = build_nc()
    return _NC_CACHE['nc']


def make_in_maps(x, w1, b1, w2, b2):
    hc = make_host_consts()
    x = np.ascontiguousarray(x, dtype=np.float32)
    in_maps = []
    for k in range(NCORES):
        xk = np.ascontiguousarray(x[:, BLK * k:BLK * (k + 1)])
        wk = make_weight_consts(w1[k], b1[k, :, 0, 0, :], w2[k], b2[k, :, 0, 0, :])
        b2k = b2[k, :, 0, 0, :]
        b2cols = np.stack([LAM - b2k[:, 0], -LAM - b2k[:, 0],
                           LAM - b2k[:, 1], -LAM - b2k[:, 1]], axis=1).astype(np.float32)
        b1cols = np.ascontiguousarray(b1[k, :, 0, 0, :], dtype=np.float32)
        m = dict(
            b1cols=b1cols,
            b2cols=b2cols,
            x32=xk,
            xbf=xk.astype(BF16),
            chpack=hc['chpack'], r1=hc['r1'], r2=hc['r2'],
            chipack=hc['chipack'], nchi=hc['nchi'], gc=hc['gc'], gs=hc['gs'],
            **wk,
        )
        in_maps.append(m)
    return in_maps


def kernel(x, w1, b1, w2, b2):
    from concourse.bass_utils import run_bass_kernel_spmd
    nc = _get_nc()
    in_maps = make_in_maps(np.asarray(x), np.asarray(w1), np.asarray(b1),
                           np.asarray(w2), np.asarray(b2))
    res = run_bass_kernel_spmd(nc, in_maps, core_ids=list(range(NCORES)))
    outs = [res.results[k]['out'] for k in range(NCORES)]
    return np.concatenate(outs, axis=1)



# revision 2
# speedup vs baseline: 1.3325x; 1.3325x over previous
"""Trainium2 Bass kernel for DistributedAFNO2D (v2).

Problem: x(2,768,256,256) f32; per-block (8 blocks of 96 ch) spectral MLP:
  out = irfft2( softshrink( W2*relu(W1*rfft2(x) + b1) + b2 ) ) + x
Block-diagonal channel mixing with shared-per-(u,v) complex 96x96 weights.

Sharding: block k -> core k (8 cores). No collectives. Each core handles
(2, 96, 256, 256) with its own block weights.

v2 layout: x host-reordered to [B, C, 128p, 2j, 256w] (h = 128*j + p) so each
channel moves with ONE dma. zbuf/sbuf_d are [B, C, 128p, 2j, 258] (u = 128*j+p).

Dataflow per core, per batch b:
  Phase A (per channel c): 2D DFT as dense matmuls
    S1 contract h: psY[w_chunk, (Yr-u256 | Yi-u256)] via CHpack
    S2 contract w: psZ[u_chunk, (Zr-v129 | Zi-v129)] via R1/R2 -> zbuf (1 dma)
  Phase B (per j, per triple of u): block MLP, W stationary, N=G*129 streams
    mix1 4 matmuls (psR/psI) + relu(+b1) on ACT -> o1r/o1i
    mix2 4 matmuls; DVE adds (b2-lam) -> t' tiles (softshrink deferred)
    v=0 cols of t' collected to dc_sb; per (b,j): softshrink + dma-transpose
    -> dct[j][ri] [128u, 96c] SBUF tiles for the DC term
  Phase C: per batch: psq[h, (hc,c)] = DC ifft via 8 matmuls N=96 -> q0sb/16
    per channel c: load t', softshrink in bf16 (s = t' - clamp(t', -2lam, 0)),
    psa/psb (ifft over u), qr/qi, pso (irfft over v via Gc/Gs),
    out = x_bf16 + q0 + pso (STT) -> 1 dma
"""
import os
import sys
import numpy as np

sys.path.insert(0, "/opt/trn_rl_repo")

import ml_dtypes

BF16 = ml_dtypes.bfloat16

H = 256
W = 256
NV = W // 2 + 1  # 129
BLK = 96
NCORES = 8
B = 2
LAM = 0.01


def make_host_consts():
    """All packed constant matrices (numpy bf16) via probing np.fft."""
    I = np.eye(H, dtype=np.float64)
    F = np.fft.fft(I, axis=0, norm='ortho')       # F[u,h]; F@x = fft(x)
    Fi = np.fft.ifft(I, axis=0, norm='ortho')     # Fi[h,u]
    CHr = F.real.T.copy()                          # [h,u]
    CHi = F.imag.T.copy()
    EWr = F.real.T[:, :NV].copy()                  # [w,v]
    EWi = F.imag.T[:, :NV].copy()
    CHIr = Fi.real.T.copy()                        # [u,h]
    CHIi = Fi.imag.T.copy()
    Ir = np.eye(NV)
    Gc = np.fft.irfft(Ir, n=W, axis=-1, norm='ortho')        # [v,w]
    Gs = np.fft.irfft(1j * Ir, n=W, axis=-1, norm='ortho')   # [v,w]

    c = {}
    # CHpack [2][128, 512]: rows h (chunk), cols [CHr-u | CHi-u]
    c['chpack'] = np.stack([
        np.concatenate([CHr[j * 128:(j + 1) * 128, :], CHi[j * 128:(j + 1) * 128, :]], axis=1)
        for j in range(2)])
    # R1 [2][128, 258] = [EWr | EWi]; R2 = [-EWi | EWr] rows w chunk
    c['r1'] = np.stack([
        np.concatenate([EWr[j * 128:(j + 1) * 128], EWi[j * 128:(j + 1) * 128]], axis=1)
        for j in range(2)])
    c['r2'] = np.stack([
        np.concatenate([-EWi[j * 128:(j + 1) * 128], EWr[j * 128:(j + 1) * 128]], axis=1)
        for j in range(2)])
    # CHIpack [2][128, 512]: rows u chunk, cols [CHIr-h | CHIi-h]
    c['chipack'] = np.stack([
        np.concatenate([CHIr[j * 128:(j + 1) * 128], CHIi[j * 128:(j + 1) * 128]], axis=1)
        for j in range(2)])
    # NCHI [2][128, 256] = -CHIi rows u chunk
    c['nchi'] = np.stack([-CHIi[j * 128:(j + 1) * 128] for j in range(2)])
    # G tiles rows v=1..128
    c['gc'] = Gc[1:129]
    c['gs'] = Gs[1:129]
    return {k: v.astype(BF16) for k, v in c.items()}


def make_weight_consts(w1k, w2k):
    """w1k/w2k: (96, 96, 2) [i, o, ri]."""
    return {
        'w1r': w1k[..., 0].astype(BF16),
        'w1i': w1k[..., 1].astype(BF16),
        'w1in': (-w1k[..., 1]).astype(BF16),
        'w2r': w2k[..., 0].astype(BF16),
        'w2i': w2k[..., 1].astype(BF16),
        'w2in': (-w2k[..., 1]).astype(BF16),
    }


def build_nc():
    import concourse.bass as bass
    import concourse.tile as tile
    from concourse import bacc, mybir

    dt = mybir.dt
    nc = bacc.Bacc("TRN2", target_bir_lowering=False, debug=False)

    # I/O  (x reordered on host: [B, BLK, 128p, 2j, 256w], h = 128*j + p)
    xbf = nc.dram_tensor("xbf", [B, BLK, 128, 2, W], dt.bfloat16, kind="ExternalInput").ap()
    chpack = nc.dram_tensor("chpack", [2, 128, 512], dt.bfloat16, kind="ExternalInput").ap()
    r1 = nc.dram_tensor("r1", [2, 128, 258], dt.bfloat16, kind="ExternalInput").ap()
    r2 = nc.dram_tensor("r2", [2, 128, 258], dt.bfloat16, kind="ExternalInput").ap()
    chipack = nc.dram_tensor("chipack", [2, 128, 512], dt.bfloat16, kind="ExternalInput").ap()
    nchi = nc.dram_tensor("nchi", [2, 128, 256], dt.bfloat16, kind="ExternalInput").ap()
    gc = nc.dram_tensor("gc", [128, 256], dt.bfloat16, kind="ExternalInput").ap()
    gs = nc.dram_tensor("gs", [128, 256], dt.bfloat16, kind="ExternalInput").ap()
    wts = {n: nc.dram_tensor(n, [96, 96], dt.bfloat16, kind="ExternalInput").ap()
           for n in ['w1r', 'w1i', 'w1in', 'w2r', 'w2i', 'w2in']}
    b1cols = nc.dram_tensor("b1cols", [96, 2], dt.float32, kind="ExternalInput").ap()
    b2cols = nc.dram_tensor("b2cols", [96, 2], dt.float32, kind="ExternalInput").ap()
    out = nc.dram_tensor("out", [B, BLK, 128, 2, W], dt.float32, kind="ExternalOutput").ap()

    # DRAM scratch: u = 128*j + p
    zbuf = nc.dram_tensor("zbuf", [B, BLK, 128, 2, 258], dt.bfloat16).ap()
    sbuf_d = nc.dram_tensor("sbufd", [B, BLK, 128, 2, 258], dt.bfloat16).ap()

    G0 = 3  # u rows per phase-B group
    TLAM = 2.0 * LAM

    with tile.TileContext(nc) as tc:
        from contextlib import ExitStack
        with ExitStack() as ctx:
            consts = ctx.enter_context(tc.tile_pool(name="consts", bufs=1))
            pa = ctx.enter_context(tc.tile_pool(name="pa", bufs=3))
            pb = ctx.enter_context(tc.tile_pool(name="pb", bufs=3))
            pc = ctx.enter_context(tc.tile_pool(name="pc", bufs=3))
            # PSUM: 4 tags x 2 bufs x 2KB = 8 banks
            psum = ctx.enter_context(tc.tile_pool(name="psum", bufs=2, space="PSUM"))

            # ---- Load constants ----
            def chunked_const(name, ap_, ncols):
                ts = []
                for j in range(2):
                    t = consts.tile([128, ncols], dt.bfloat16, tag=f"{name}{j}", name=f"{name}{j}")
                    nc.sync.dma_start(out=t, in_=ap_[j])
                    ts.append(t)
                return ts

            t_ch = chunked_const("t_ch", chpack, 512)
            t_r1 = chunked_const("t_r1", r1, 258)
            t_r2 = chunked_const("t_r2", r2, 258)
            t_chi = chunked_const("t_chi", chipack, 512)
            t_nchi = chunked_const("t_nchi", nchi, 256)
            t_gc = consts.tile([128, 256], dt.bfloat16, tag="t_gc", name="t_gc")
            nc.sync.dma_start(out=t_gc, in_=gc)
            t_gs = consts.tile([128, 256], dt.bfloat16, tag="t_gs", name="t_gs")
            nc.sync.dma_start(out=t_gs, in_=gs)
            t_w = {}
            for n, ap_ in wts.items():
                t_w[n] = consts.tile([96, 96], dt.bfloat16, tag=f"t_{n}", name=f"t_{n}")
                nc.sync.dma_start(out=t_w[n], in_=ap_)

            t_b1 = consts.tile([96, 2], dt.float32, tag="t_b1", name="t_b1")
            nc.sync.dma_start(out=t_b1, in_=b1cols)
            t_b2 = consts.tile([96, 2], dt.float32, tag="t_b2", name="t_b2")
            nc.sync.dma_start(out=t_b2, in_=b2cols)

            for b in range(B):
                # ================= Phase A =================
                for c in range(BLK):
                    xt = pa.tile([128, 2, 256], dt.bfloat16, tag="xt", name="xt")
                    nc.gpsimd.dma_start(out=xt, in_=xbf[b, c])

                    ys = []
                    for wc in range(2):
                        psy = psum.tile([128, 512], dt.float32, tag="pA", name="psy")
                        nc.tensor.matmul(psy, lhsT=xt[:, 0, wc * 128:(wc + 1) * 128],
                                         rhs=t_ch[0], start=True, stop=False)
                        nc.tensor.matmul(psy, lhsT=xt[:, 1, wc * 128:(wc + 1) * 128],
                                         rhs=t_ch[1], start=False, stop=True)
                        y = pa.tile([128, 512], dt.bfloat16, tag=f"y{wc}", name=f"y{wc}")
                        if wc == 0:
                            nc.scalar.copy(y, psy)
                        else:
                            nc.vector.tensor_scalar_add(y, psy, 0.0)
                        ys.append(y)

                    zt2 = pa.tile([128, 2, 258], dt.bfloat16, tag="zt2", name="zt2")
                    for uc in range(2):
                        psz = psum.tile([128, 258], dt.float32, tag="pB", name="psz")
                        us = slice(uc * 128, (uc + 1) * 128)
                        us2 = slice(256 + uc * 128, 256 + (uc + 1) * 128)
                        nc.tensor.matmul(psz, lhsT=ys[0][:, us], rhs=t_r1[0], start=True, stop=False)
                        nc.tensor.matmul(psz, lhsT=ys[0][:, us2], rhs=t_r2[0], start=False, stop=False)
                        nc.tensor.matmul(psz, lhsT=ys[1][:, us], rhs=t_r1[1], start=False, stop=False)
                        nc.tensor.matmul(psz, lhsT=ys[1][:, us2], rhs=t_r2[1], start=False, stop=True)
                        if uc == 0:
                            nc.vector.tensor_scalar_add(zt2[:, 0, :], psz, 0.0)
                        else:
                            nc.scalar.copy(zt2[:, 1, :], psz)
                    nc.sync.dma_start(out=zbuf[b, c], in_=zt2)

                # ================= Phase B =================
                dct = {}
                for j in range(2):
                    dc_sb = pb.tile([96, 2, 128], dt.bfloat16, tag="dc_sb", name="dc_sb")
                    p0 = 0
                    while p0 < 128:
                        G = min(G0, 128 - p0)
                        NG = G * 129
                        zt = pb.tile([96, G0, 258], dt.bfloat16, tag="zt", name="zt")
                        nc.gpsimd.dma_start(out=zt[:, 0:G, :], in_=zbuf[b, :, p0:p0 + G, j, :])
                        zr = zt[:, 0:G, 0:129]
                        zi = zt[:, 0:G, 129:258]

                        psR = psum.tile([96, G0 * 129], dt.float32, tag="pA", name="psR")
                        psI = psum.tile([96, G0 * 129], dt.float32, tag="pB", name="psI")
                        nc.tensor.matmul(psR[:, 0:NG], lhsT=t_w['w1r'], rhs=zr, start=True, stop=False)
                        nc.tensor.matmul(psR[:, 0:NG], lhsT=t_w['w1in'], rhs=zi, start=False, stop=True)
                        nc.tensor.matmul(psI[:, 0:NG], lhsT=t_w['w1i'], rhs=zr, start=True, stop=False)
                        nc.tensor.matmul(psI[:, 0:NG], lhsT=t_w['w1r'], rhs=zi, start=False, stop=True)

                        o1r = pb.tile([96, G0 * 129], dt.bfloat16, tag="o1r", name="o1r")
                        o1i = pb.tile([96, G0 * 129], dt.bfloat16, tag="o1i", name="o1i")
                        nc.scalar.activation(o1r[:, 0:NG], psR[:, 0:NG],
                                             mybir.ActivationFunctionType.Relu, bias=t_b1[:, 0:1])
                        nc.scalar.activation(o1i[:, 0:NG], psI[:, 0:NG],
                                             mybir.ActivationFunctionType.Relu, bias=t_b1[:, 1:2])

                        psR2 = psum.tile([96, G0 * 129], dt.float32, tag="pC", name="psR2")
                        psI2 = psum.tile([96, G0 * 129], dt.float32, tag="pD", name="psI2")
                        nc.tensor.matmul(psR2[:, 0:NG], lhsT=t_w['w2r'], rhs=o1r[:, 0:NG], start=True, stop=False)
                        nc.tensor.matmul(psR2[:, 0:NG], lhsT=t_w['w2in'], rhs=o1i[:, 0:NG], start=False, stop=True)
                        nc.tensor.matmul(psI2[:, 0:NG], lhsT=t_w['w2i'], rhs=o1r[:, 0:NG], start=True, stop=False)
                        nc.tensor.matmul(psI2[:, 0:NG], lhsT=t_w['w2r'], rhs=o1i[:, 0:NG], start=False, stop=True)

                        # t' = o2 + b2 - lam  (softshrink deferred to phase C)
                        tr_ = pb.tile([96, G0, 129], dt.bfloat16, tag="tr_", name="tr_")
                        ti_ = pb.tile([96, G0, 129], dt.bfloat16, tag="ti_", name="ti_")
                        nc.vector.tensor_scalar_add(tr_[:, 0:G, :], psR2[:, 0:NG], t_b2[:, 0:1])
                        nc.vector.tensor_scalar_add(ti_[:, 0:G, :], psI2[:, 0:NG], t_b2[:, 1:2])
                        nc.sync.dma_start(out=sbuf_d[b, :, p0:p0 + G, j, 0:129], in_=tr_[:, 0:G, :])
                        nc.sync.dma_start(out=sbuf_d[b, :, p0:p0 + G, j, 129:258], in_=ti_[:, 0:G, :])

                        # collect v=0 cols for the DC term
                        nc.vector.tensor_scalar_add(dc_sb[:, 0, p0:p0 + G], tr_[:, 0:G, 0], 0.0)
                        nc.vector.tensor_scalar_add(dc_sb[:, 1, p0:p0 + G], ti_[:, 0:G, 0], 0.0)
                        p0 += G

                    # softshrink the DC cols, transpose to [128u, 96c]
                    dccl = pb.tile([96, 2, 128], dt.bfloat16, tag="dccl", name="dccl")
                    nc.vector.tensor_scalar(dccl, dc_sb, 0.0, -TLAM,
                                            mybir.AluOpType.min, mybir.AluOpType.max)
                    dcs = pb.tile([96, 2, 128], dt.bfloat16, tag="dcs", name="dcs")
                    nc.vector.tensor_tensor(dcs, dc_sb, dccl, mybir.AluOpType.subtract)
                    for ri in range(2):
                        t = pb.tile([128, 96], dt.bfloat16, tag=f"dct{j}{ri}", name=f"dct{j}{ri}", bufs=2)
                        nc.scalar.dma_start_transpose(out=t, in_=dcs[:, ri, :])
                        dct[(j, ri)] = t

                # ================= Phase C =================
                # DC term: q0[(hc,c), h] = (1/16) ifft_u at v=0
                psq = psum.tile([128, 192], dt.float32, tag="pD", name="psq")
                for hc in range(2):
                    hs = slice(hc * 128, (hc + 1) * 128)
                    qs = slice(hc * 96, (hc + 1) * 96)
                    nc.tensor.matmul(psq[:, qs], lhsT=t_chi[0][:, hs], rhs=dct[(0, 0)],
                                     start=True, stop=False, skip_group_check=True)
                    nc.tensor.matmul(psq[:, qs], lhsT=t_nchi[0][:, hs], rhs=dct[(0, 1)],
                                     start=False, stop=False, skip_group_check=True)
                    nc.tensor.matmul(psq[:, qs], lhsT=t_chi[1][:, hs], rhs=dct[(1, 0)],
                                     start=False, stop=False, skip_group_check=True)
                    nc.tensor.matmul(psq[:, qs], lhsT=t_nchi[1][:, hs], rhs=dct[(1, 1)],
                                     start=False, stop=True, skip_group_check=True)
                q0sb = pc.tile([128, 192], dt.float32, tag="q0sb", name="q0sb")
                nc.vector.tensor_scalar_mul(q0sb, psq, 1.0 / 16.0)

                for c in range(BLK):
                    st2 = pc.tile([128, 2, 258], dt.bfloat16, tag="st2", name="st2")
                    nc.gpsimd.dma_start(out=st2, in_=sbuf_d[b, c])
                    # softshrink: s = t' - clamp(t', -2lam, 0)   (bf16, 2x mode)
                    cl2 = pc.tile([128, 2, 258], dt.bfloat16, tag="cl2", name="cl2")
                    nc.gpsimd.tensor_scalar(cl2, st2, 0.0, -TLAM,
                                            mybir.AluOpType.min, mybir.AluOpType.max)
                    s2 = pc.tile([128, 2, 258], dt.bfloat16, tag="s2", name="s2")
                    nc.vector.tensor_tensor(s2, st2, cl2, mybir.AluOpType.subtract)

                    # QrT/QiT = ifft over u (contraction 256u, complex)
                    psa = psum.tile([128, 256], dt.float32, tag="pA", name="psa")
                    nc.tensor.matmul(psa, lhsT=s2[:, 0, 1:129], rhs=t_chi[0][:, 0:256], start=True, stop=False)
                    nc.tensor.matmul(psa, lhsT=s2[:, 1, 1:129], rhs=t_chi[1][:, 0:256], start=False, stop=False)
                    nc.tensor.matmul(psa, lhsT=s2[:, 0, 130:258], rhs=t_nchi[0], start=False, stop=False)
                    nc.tensor.matmul(psa, lhsT=s2[:, 1, 130:258], rhs=t_nchi[1], start=False, stop=True)
                    psb = psum.tile([128, 256], dt.float32, tag="pB", name="psb")
                    nc.tensor.matmul(psb, lhsT=s2[:, 0, 1:129], rhs=t_chi[0][:, 256:512], start=True, stop=False)
                    nc.tensor.matmul(psb, lhsT=s2[:, 1, 1:129], rhs=t_chi[1][:, 256:512], start=False, stop=False)
                    nc.tensor.matmul(psb, lhsT=s2[:, 0, 130:258], rhs=t_chi[0][:, 0:256], start=False, stop=False)
                    nc.tensor.matmul(psb, lhsT=s2[:, 1, 130:258], rhs=t_chi[1][:, 0:256], start=False, stop=True)

                    qr = pc.tile([128, 256], dt.bfloat16, tag="qr", name="qr")
                    nc.scalar.copy(qr, psa)
                    qi = pc.tile([128, 256], dt.bfloat16, tag="qi", name="qi")
                    nc.scalar.copy(qi, psb)

                    xt_r = pc.tile([128, 2, 256], dt.bfloat16, tag="xt_r", name="xt_r")
                    nc.gpsimd.dma_start(out=xt_r, in_=xbf[b, c])
                    ot = pc.tile([128, 2, 256], dt.float32, tag="ot", name="ot")
                    for hc in range(2):
                        hs = slice(hc * 128, (hc + 1) * 128)
                        pso = psum.tile([128, 256], dt.float32, tag="pC", name="pso")
                        nc.tensor.matmul(pso, lhsT=qr[:, hs], rhs=t_gc, start=True, stop=False)
                        nc.tensor.matmul(pso, lhsT=qi[:, hs], rhs=t_gs, start=False, stop=True)
                        nc.vector.scalar_tensor_tensor(
                            ot[:, hc, :], xt_r[:, hc, :], q0sb[:, hc * 96 + c:hc * 96 + c + 1], pso,
                            mybir.AluOpType.add, mybir.AluOpType.add)
                    nc.sync.dma_start(out=out[b, c], in_=ot)
    nc.compile()
    return nc


_NC_CACHE = {}


def _get_nc():
    if 'nc' not in _NC_CACHE:
        _NC_CACHE['nc'] = build_nc()
    return _NC_CACHE['nc']


def make_in_maps(x, w1, b1, w2, b2):
    hc = make_host_consts()
    x = np.ascontiguousarray(x, dtype=np.float32)
    in_maps = []
    for k in range(NCORES):
        # [B, 96, 256, 256] -> [B, 96, 128p, 2j, 256w], h = 128*j + p
        xk = x[:, BLK * k:BLK * (k + 1)].reshape(B, BLK, 2, 128, W).transpose(0, 1, 3, 2, 4)
        wk = make_weight_consts(w1[k], w2[k])
        b1k = b1[k, :, 0, 0, :]
        b2k = b2[k, :, 0, 0, :]
        m = dict(
            b1cols=np.ascontiguousarray(b1k, dtype=np.float32),
            b2cols=np.ascontiguousarray(b2k - LAM, dtype=np.float32),
            xbf=np.ascontiguousarray(xk).astype(BF16),
            chpack=hc['chpack'], r1=hc['r1'], r2=hc['r2'],
            chipack=hc['chipack'], nchi=hc['nchi'], gc=hc['gc'], gs=hc['gs'],
            **wk,
        )
        in_maps.append(m)
    return in_maps


def postprocess(outs):
    """outs: list of [B, 96, 128, 2, 256] per core -> [B, 768, 256, 256]."""
    full = np.concatenate(outs, axis=1)
    return np.ascontiguousarray(
        full.transpose(0, 1, 3, 2, 4).reshape(B, BLK * NCORES, H, W))


def kernel(x, w1, b1, w2, b2):
    from concourse.bass_utils import run_bass_kernel_spmd
    nc = _get_nc()
    in_maps = make_in_maps(np.asarray(x), np.asarray(w1), np.asarray(b1),
                           np.asarray(w2), np.asarray(b2))
    res = run_bass_kernel_spmd(nc, in_maps, core_ids=list(range(NCORES)))
    return postprocess([res.results[k]['out'] for k in range(NCORES)])


# revision 3
# speedup vs baseline: 1.4441x; 1.0838x over previous
"""Trainium2 Bass kernel for DistributedAFNO2D (v3).

Problem: x(2,768,256,256) f32; per-block (8 blocks of 96 ch) spectral MLP:
  out = irfft2( softshrink( W2*relu(W1*rfft2(x) + b1) + b2 ) ) + x
Block-diagonal channel mixing with shared-per-(u,v) complex 96x96 weights.

Sharding: block k -> core k (8 cores). No collectives. Each core handles
(2, 96, 256, 256) with its own block weights.

v3 layout: x host-reordered to [B, C, 128p, 2j, 256w] (h = 128*j + p) so each
channel moves with ONE dma. zbuf/sbuf_d are [B, C, 128p, 2j, 258] (u = 128*j+p).

Dataflow per core, per batch b:
  Phase A (per channel c): 2D DFT as dense matmuls
    S1 contract h: psY[w_chunk, (Yr-u256 | Yi-u256)] via CHpack
    S2 contract w: psZ[u_chunk, (Zr-v129 | Zi-v129)] via R1/R2 -> zbuf (1 dma)
  Phase B (per j, per triple of u): block MLP, W stationary, N=G*129 streams
    mix1 4 matmuls (psR/psI) + relu(+b1) on ACT -> o1r/o1i
    mix2 4 matmuls; DVE adds (b2-lam) -> t' tiles (softshrink deferred)
    v=0 cols of t' collected to dc_sb; per (b,j): softshrink + dma-transpose
    -> dct[j][ri] [128u, 96c] SBUF tiles for the DC term
  Phase C: per batch: psq[h, (hc,c)] = DC ifft via 8 matmuls N=96 -> q0sb/16
    per channel c: load t', softshrink in bf16 (s = t' - clamp(t', -2lam, 0)),
    psab = [QrT | QiT] (ifft over u, 4 matmuls N=512 w/ [-CHIi|CHIr] pack),
    qrqi copy, pso (irfft over v via Gc/Gs), out = x_bf16 + q0 + pso -> 1 dma
  C(b=0) is interleaved channel-by-channel with A(b=1) to overlap engine mixes.
"""
import os
import sys
import numpy as np

sys.path.insert(0, "/opt/trn_rl_repo")

import ml_dtypes

BF16 = ml_dtypes.bfloat16

H = 256
W = 256
NV = W // 2 + 1  # 129
BLK = 96
NCORES = 8
B = 2
LAM = 0.01


def make_host_consts():
    """All packed constant matrices (numpy bf16) via probing np.fft."""
    I = np.eye(H, dtype=np.float64)
    F = np.fft.fft(I, axis=0, norm='ortho')       # F[u,h]; F@x = fft(x)
    Fi = np.fft.ifft(I, axis=0, norm='ortho')     # Fi[h,u]
    CHr = F.real.T.copy()                          # [h,u]
    CHi = F.imag.T.copy()
    EWr = F.real.T[:, :NV].copy()                  # [w,v]
    EWi = F.imag.T[:, :NV].copy()
    CHIr = Fi.real.T.copy()                        # [u,h]
    CHIi = Fi.imag.T.copy()
    Ir = np.eye(NV)
    Gc = np.fft.irfft(Ir, n=W, axis=-1, norm='ortho')        # [v,w]
    Gs = np.fft.irfft(1j * Ir, n=W, axis=-1, norm='ortho')   # [v,w]

    c = {}
    # CHpack [2][128, 512]: rows h (chunk), cols [CHr-u | CHi-u]
    c['chpack'] = np.stack([
        np.concatenate([CHr[j * 128:(j + 1) * 128, :], CHi[j * 128:(j + 1) * 128, :]], axis=1)
        for j in range(2)])
    # R1 [2][128, 258] = [EWr | EWi]; R2 = [-EWi | EWr] rows w chunk
    c['r1'] = np.stack([
        np.concatenate([EWr[j * 128:(j + 1) * 128], EWi[j * 128:(j + 1) * 128]], axis=1)
        for j in range(2)])
    c['r2'] = np.stack([
        np.concatenate([-EWi[j * 128:(j + 1) * 128], EWr[j * 128:(j + 1) * 128]], axis=1)
        for j in range(2)])
    # CHIpack [2][128, 512]: rows u chunk, cols [CHIr-h | CHIi-h]
    c['chipack'] = np.stack([
        np.concatenate([CHIr[j * 128:(j + 1) * 128], CHIi[j * 128:(j + 1) * 128]], axis=1)
        for j in range(2)])
    # NCHICHI [2][128, 512]: cols [-CHIi-h | CHIr-h] (for merged psab)
    c['nchichi'] = np.stack([
        np.concatenate([-CHIi[j * 128:(j + 1) * 128], CHIr[j * 128:(j + 1) * 128]], axis=1)
        for j in range(2)])
    # NCHI [2][128, 256] = -CHIi rows u chunk (DC term)
    c['nchi'] = np.stack([-CHIi[j * 128:(j + 1) * 128] for j in range(2)])
    # G tiles rows v=1..128
    c['gc'] = Gc[1:129]
    c['gs'] = Gs[1:129]
    return {k: v.astype(BF16) for k, v in c.items()}


def make_weight_consts(w1k, w2k):
    """w1k/w2k: (96, 96, 2) [i, o, ri]."""
    return {
        'w1r': w1k[..., 0].astype(BF16),
        'w1i': w1k[..., 1].astype(BF16),
        'w1in': (-w1k[..., 1]).astype(BF16),
        'w2r': w2k[..., 0].astype(BF16),
        'w2i': w2k[..., 1].astype(BF16),
        'w2in': (-w2k[..., 1]).astype(BF16),
    }


def build_nc():
    import concourse.bass as bass
    import concourse.tile as tile
    from concourse import bacc, mybir

    dt = mybir.dt
    nc = bacc.Bacc("TRN2", target_bir_lowering=False, debug=False)

    # I/O  (x reordered on host: [B, BLK, 128p, 2j, 256w], h = 128*j + p)
    xbf = nc.dram_tensor("xbf", [B, BLK, 128, 2, W], dt.bfloat16, kind="ExternalInput").ap()
    chpack = nc.dram_tensor("chpack", [2, 128, 512], dt.bfloat16, kind="ExternalInput").ap()
    r1 = nc.dram_tensor("r1", [2, 128, 258], dt.bfloat16, kind="ExternalInput").ap()
    r2 = nc.dram_tensor("r2", [2, 128, 258], dt.bfloat16, kind="ExternalInput").ap()
    chipack = nc.dram_tensor("chipack", [2, 128, 512], dt.bfloat16, kind="ExternalInput").ap()
    nchichi = nc.dram_tensor("nchichi", [2, 128, 512], dt.bfloat16, kind="ExternalInput").ap()
    nchi = nc.dram_tensor("nchi", [2, 128, 256], dt.bfloat16, kind="ExternalInput").ap()
    gc = nc.dram_tensor("gc", [128, 256], dt.bfloat16, kind="ExternalInput").ap()
    gs = nc.dram_tensor("gs", [128, 256], dt.bfloat16, kind="ExternalInput").ap()
    wts = {n: nc.dram_tensor(n, [96, 96], dt.bfloat16, kind="ExternalInput").ap()
           for n in ['w1r', 'w1i', 'w1in', 'w2r', 'w2i', 'w2in']}
    b1cols = nc.dram_tensor("b1cols", [96, 2], dt.float32, kind="ExternalInput").ap()
    b2cols = nc.dram_tensor("b2cols", [96, 2], dt.float32, kind="ExternalInput").ap()
    out = nc.dram_tensor("out", [B, BLK, 128, 2, W], dt.float32, kind="ExternalOutput").ap()

    # DRAM scratch: u = 128*j + p
    zbuf = nc.dram_tensor("zbuf", [B, BLK, 128, 2, 258], dt.bfloat16).ap()
    sbuf_d = nc.dram_tensor("sbufd", [B, BLK, 128, 2, 258], dt.bfloat16).ap()

    G0 = 3  # u rows per phase-B group
    TLAM = 2.0 * LAM

    with tile.TileContext(nc) as tc:
        from contextlib import ExitStack
        with ExitStack() as ctx:
            consts = ctx.enter_context(tc.tile_pool(name="consts", bufs=1))
            pa = ctx.enter_context(tc.tile_pool(name="pa", bufs=3))
            pb = ctx.enter_context(tc.tile_pool(name="pb", bufs=3))
            pc = ctx.enter_context(tc.tile_pool(name="pc", bufs=3))
            # PSUM: 4 tags x 2 bufs x 2KB = 8 banks
            psum = ctx.enter_context(tc.tile_pool(name="psum", bufs=2, space="PSUM"))

            # ---- Load constants ----
            def chunked_const(name, ap_, ncols):
                ts = []
                for j in range(2):
                    t = consts.tile([128, ncols], dt.bfloat16, tag=f"{name}{j}", name=f"{name}{j}")
                    nc.sync.dma_start(out=t, in_=ap_[j])
                    ts.append(t)
                return ts

            t_ch = chunked_const("t_ch", chpack, 512)
            t_r1 = chunked_const("t_r1", r1, 258)
            t_r2 = chunked_const("t_r2", r2, 258)
            t_chi = chunked_const("t_chi", chipack, 512)
            t_ncc = chunked_const("t_ncc", nchichi, 512)
            t_nchi = chunked_const("t_nchi", nchi, 256)
            t_gc = consts.tile([128, 256], dt.bfloat16, tag="t_gc", name="t_gc")
            nc.sync.dma_start(out=t_gc, in_=gc)
            t_gs = consts.tile([128, 256], dt.bfloat16, tag="t_gs", name="t_gs")
            nc.sync.dma_start(out=t_gs, in_=gs)
            t_w = {}
            for n, ap_ in wts.items():
                t_w[n] = consts.tile([96, 96], dt.bfloat16, tag=f"t_{n}", name=f"t_{n}")
                nc.sync.dma_start(out=t_w[n], in_=ap_)

            t_b1 = consts.tile([96, 2], dt.float32, tag="t_b1", name="t_b1")
            nc.sync.dma_start(out=t_b1, in_=b1cols)
            t_b2 = consts.tile([96, 2], dt.float32, tag="t_b2", name="t_b2")
            nc.sync.dma_start(out=t_b2, in_=b2cols)

            # ---------- phase helpers (emit per-channel work) ----------
            def phaseA_ch(b, c):
                xt = pa.tile([128, 2, 256], dt.bfloat16, tag="xt", name="xt")
                nc.gpsimd.dma_start(out=xt, in_=xbf[b, c])

                ys = []
                for wc in range(2):
                    psy = psum.tile([128, 512], dt.float32, tag="pA", name="psy")
                    nc.tensor.matmul(psy, lhsT=xt[:, 0, wc * 128:(wc + 1) * 128],
                                     rhs=t_ch[0], start=True, stop=False)
                    nc.tensor.matmul(psy, lhsT=xt[:, 1, wc * 128:(wc + 1) * 128],
                                     rhs=t_ch[1], start=False, stop=True)
                    y = pa.tile([128, 512], dt.bfloat16, tag=f"y{wc}", name=f"y{wc}")
                    if wc == 0:
                        nc.scalar.copy(y, psy)
                    else:
                        nc.vector.tensor_scalar_add(y, psy, 0.0)
                    ys.append(y)

                zt2 = pa.tile([128, 2, 258], dt.bfloat16, tag="zt2", name="zt2")
                for uc in range(2):
                    psz = psum.tile([128, 258], dt.float32, tag="pB", name="psz")
                    us = slice(uc * 128, (uc + 1) * 128)
                    us2 = slice(256 + uc * 128, 256 + (uc + 1) * 128)
                    nc.tensor.matmul(psz, lhsT=ys[0][:, us], rhs=t_r1[0], start=True, stop=False)
                    nc.tensor.matmul(psz, lhsT=ys[0][:, us2], rhs=t_r2[0], start=False, stop=False)
                    nc.tensor.matmul(psz, lhsT=ys[1][:, us], rhs=t_r1[1], start=False, stop=False)
                    nc.tensor.matmul(psz, lhsT=ys[1][:, us2], rhs=t_r2[1], start=False, stop=True)
                    if uc == 0:
                        nc.vector.tensor_scalar_add(zt2[:, 0, :], psz, 0.0)
                    else:
                        nc.scalar.copy(zt2[:, 1, :], psz)
                nc.sync.dma_start(out=zbuf[b, c], in_=zt2)

            def phaseB(b):
                dct = {}
                for j in range(2):
                    dc_sb = pb.tile([96, 2, 128], dt.bfloat16, tag="dc_sb", name="dc_sb")
                    p0 = 0
                    while p0 < 128:
                        G = min(G0, 128 - p0)
                        NG = G * 129
                        zt = pb.tile([96, G0, 258], dt.bfloat16, tag="zt", name="zt")
                        nc.gpsimd.dma_start(out=zt[:, 0:G, :], in_=zbuf[b, :, p0:p0 + G, j, :])
                        zr = zt[:, 0:G, 0:129]
                        zi = zt[:, 0:G, 129:258]

                        psR = psum.tile([96, G0 * 129], dt.float32, tag="pA", name="psR")
                        psI = psum.tile([96, G0 * 129], dt.float32, tag="pB", name="psI")
                        nc.tensor.matmul(psR[:, 0:NG], lhsT=t_w['w1r'], rhs=zr, start=True, stop=False)
                        nc.tensor.matmul(psR[:, 0:NG], lhsT=t_w['w1in'], rhs=zi, start=False, stop=True)
                        nc.tensor.matmul(psI[:, 0:NG], lhsT=t_w['w1i'], rhs=zr, start=True, stop=False)
                        nc.tensor.matmul(psI[:, 0:NG], lhsT=t_w['w1r'], rhs=zi, start=False, stop=True)

                        o1r = pb.tile([96, G0 * 129], dt.bfloat16, tag="o1r", name="o1r")
                        o1i = pb.tile([96, G0 * 129], dt.bfloat16, tag="o1i", name="o1i")
                        nc.scalar.activation(o1r[:, 0:NG], psR[:, 0:NG],
                                             mybir.ActivationFunctionType.Relu, bias=t_b1[:, 0:1])
                        nc.scalar.activation(o1i[:, 0:NG], psI[:, 0:NG],
                                             mybir.ActivationFunctionType.Relu, bias=t_b1[:, 1:2])

                        psR2 = psum.tile([96, G0 * 129], dt.float32, tag="pC", name="psR2")
                        psI2 = psum.tile([96, G0 * 129], dt.float32, tag="pD", name="psI2")
                        nc.tensor.matmul(psR2[:, 0:NG], lhsT=t_w['w2r'], rhs=o1r[:, 0:NG], start=True, stop=False)
                        nc.tensor.matmul(psR2[:, 0:NG], lhsT=t_w['w2in'], rhs=o1i[:, 0:NG], start=False, stop=True)
                        nc.tensor.matmul(psI2[:, 0:NG], lhsT=t_w['w2i'], rhs=o1r[:, 0:NG], start=True, stop=False)
                        nc.tensor.matmul(psI2[:, 0:NG], lhsT=t_w['w2r'], rhs=o1i[:, 0:NG], start=False, stop=True)

                        # t' = o2 + b2 - lam  (softshrink deferred to phase C)
                        tr_ = pb.tile([96, G0, 129], dt.bfloat16, tag="tr_", name="tr_")
                        ti_ = pb.tile([96, G0, 129], dt.bfloat16, tag="ti_", name="ti_")
                        nc.vector.tensor_scalar_add(tr_[:, 0:G, :], psR2[:, 0:NG], t_b2[:, 0:1])
                        nc.vector.tensor_scalar_add(ti_[:, 0:G, :], psI2[:, 0:NG], t_b2[:, 1:2])
                        nc.sync.dma_start(out=sbuf_d[b, :, p0:p0 + G, j, 0:129], in_=tr_[:, 0:G, :])
                        nc.sync.dma_start(out=sbuf_d[b, :, p0:p0 + G, j, 129:258], in_=ti_[:, 0:G, :])

                        # collect v=0 cols for the DC term (scalar engine copies)
                        nc.scalar.copy(dc_sb[:, 0, p0:p0 + G], tr_[:, 0:G, 0])
                        nc.scalar.copy(dc_sb[:, 1, p0:p0 + G], ti_[:, 0:G, 0])
                        p0 += G

                    # softshrink the DC cols, transpose to [128u, 96c]
                    dccl = pb.tile([96, 2, 128], dt.bfloat16, tag="dccl", name="dccl")
                    nc.vector.tensor_scalar(dccl, dc_sb, 0.0, -TLAM,
                                            mybir.AluOpType.min, mybir.AluOpType.max)
                    dcs = pb.tile([96, 2, 128], dt.bfloat16, tag="dcs", name="dcs")
                    nc.vector.tensor_tensor(dcs, dc_sb, dccl, mybir.AluOpType.subtract)
                    for ri in range(2):
                        t = pb.tile([128, 96], dt.bfloat16, tag=f"dct{j}{ri}", name=f"dct{j}{ri}", bufs=2)
                        nc.scalar.dma_start_transpose(out=t, in_=dcs[:, ri, :])
                        dct[(j, ri)] = t
                return dct

            def phaseC_start(b, dct):
                # DC term: q0[(hc,c), h] = (1/16) ifft_u at v=0
                psq = psum.tile([128, 192], dt.float32, tag="pC", name="psq")
                for hc in range(2):
                    hs = slice(hc * 128, (hc + 1) * 128)
                    qs = slice(hc * 96, (hc + 1) * 96)
                    nc.tensor.matmul(psq[:, qs], lhsT=t_chi[0][:, hs], rhs=dct[(0, 0)],
                                     start=True, stop=False, skip_group_check=True)
                    nc.tensor.matmul(psq[:, qs], lhsT=t_nchi[0][:, hs], rhs=dct[(0, 1)],
                                     start=False, stop=False, skip_group_check=True)
                    nc.tensor.matmul(psq[:, qs], lhsT=t_chi[1][:, hs], rhs=dct[(1, 0)],
                                     start=False, stop=False, skip_group_check=True)
                    nc.tensor.matmul(psq[:, qs], lhsT=t_nchi[1][:, hs], rhs=dct[(1, 1)],
                                     start=False, stop=True, skip_group_check=True)
                q0sb = pc.tile([128, 192], dt.float32, tag="q0sb", name="q0sb")
                nc.vector.tensor_scalar_mul(q0sb, psq, 1.0 / 16.0)
                return q0sb

            def phaseC_ch(b, c, q0sb):
                st2 = pc.tile([128, 2, 258], dt.bfloat16, tag="st2", name="st2")
                nc.gpsimd.dma_start(out=st2, in_=sbuf_d[b, c])
                # softshrink: s = t' - clamp(t', -2lam, 0)   (bf16, 2x mode)
                cl2 = pc.tile([128, 2, 258], dt.bfloat16, tag="cl2", name="cl2")
                nc.gpsimd.tensor_scalar(cl2, st2, 0.0, -TLAM,
                                        mybir.AluOpType.min, mybir.AluOpType.max)
                s2 = pc.tile([128, 2, 258], dt.bfloat16, tag="s2", name="s2")
                nc.vector.tensor_tensor(s2, st2, cl2, mybir.AluOpType.subtract)

                # [QrT | QiT] = ifft over u (contraction 256u, complex), merged
                psab = psum.tile([128, 512], dt.float32, tag="pD", name="psab")
                nc.tensor.matmul(psab, lhsT=s2[:, 0, 1:129], rhs=t_chi[0], start=True, stop=False)
                nc.tensor.matmul(psab, lhsT=s2[:, 1, 1:129], rhs=t_chi[1], start=False, stop=False)
                nc.tensor.matmul(psab, lhsT=s2[:, 0, 130:258], rhs=t_ncc[0], start=False, stop=False)
                nc.tensor.matmul(psab, lhsT=s2[:, 1, 130:258], rhs=t_ncc[1], start=False, stop=True)

                qrqi = pc.tile([128, 512], dt.bfloat16, tag="qrqi", name="qrqi")
                nc.scalar.copy(qrqi, psab)

                xt_r = pc.tile([128, 2, 256], dt.bfloat16, tag="xt_r", name="xt_r")
                nc.sync.dma_start(out=xt_r, in_=xbf[b, c])
                ot = pc.tile([128, 2, 256], dt.float32, tag="ot", name="ot")
                for hc in range(2):
                    hs = slice(hc * 128, (hc + 1) * 128)
                    pso = psum.tile([128, 256], dt.float32, tag="pC", name="pso")
                    nc.tensor.matmul(pso, lhsT=qrqi[:, hs], rhs=t_gc, start=True, stop=False)
                    nc.tensor.matmul(pso, lhsT=qrqi[:, 256 + hs.start:256 + hs.stop], rhs=t_gs,
                                     start=False, stop=True)
                    nc.vector.scalar_tensor_tensor(
                        ot[:, hc, :], xt_r[:, hc, :], q0sb[:, hc * 96 + c:hc * 96 + c + 1], pso,
                        mybir.AluOpType.add, mybir.AluOpType.add)
                nc.sync.dma_start(out=out[b, c], in_=ot)

            # ---------- schedule ----------
            for c in range(BLK):
                phaseA_ch(0, c)
            dct0 = phaseB(0)
            q0sb0 = phaseC_start(0, dct0)
            # interleave C(0) with A(1): complementary engine mixes
            for c in range(BLK):
                phaseC_ch(0, c, q0sb0)
                phaseA_ch(1, c)
            dct1 = phaseB(1)
            q0sb1 = phaseC_start(1, dct1)
            for c in range(BLK):
                phaseC_ch(1, c, q0sb1)
    nc.compile()
    return nc


_NC_CACHE = {}


def _get_nc():
    if 'nc' not in _NC_CACHE:
        _NC_CACHE['nc'] = build_nc()
    return _NC_CACHE['nc']


def make_in_maps(x, w1, b1, w2, b2):
    hc = make_host_consts()
    x = np.ascontiguousarray(x, dtype=np.float32)
    in_maps = []
    for k in range(NCORES):
        # [B, 96, 256, 256] -> [B, 96, 128p, 2j, 256w], h = 128*j + p
        xk = x[:, BLK * k:BLK * (k + 1)].reshape(B, BLK, 2, 128, W).transpose(0, 1, 3, 2, 4)
        wk = make_weight_consts(w1[k], w2[k])
        b1k = b1[k, :, 0, 0, :]
        b2k = b2[k, :, 0, 0, :]
        m = dict(
            b1cols=np.ascontiguousarray(b1k, dtype=np.float32),
            b2cols=np.ascontiguousarray(b2k - LAM, dtype=np.float32),
            xbf=np.ascontiguousarray(xk).astype(BF16),
            chpack=hc['chpack'], r1=hc['r1'], r2=hc['r2'],
            chipack=hc['chipack'], nchichi=hc['nchichi'], nchi=hc['nchi'],
            gc=hc['gc'], gs=hc['gs'],
            **wk,
        )
        in_maps.append(m)
    return in_maps


def postprocess(outs):
    """outs: list of [B, 96, 128, 2, 256] per core -> [B, 768, 256, 256]."""
    full = np.concatenate(outs, axis=1)
    return np.ascontiguousarray(
        full.transpose(0, 1, 3, 2, 4).reshape(B, BLK * NCORES, H, W))


def kernel(x, w1, b1, w2, b2):
    from concourse.bass_utils import run_bass_kernel_spmd
    nc = _get_nc()
    in_maps = make_in_maps(np.asarray(x), np.asarray(w1), np.asarray(b1),
                           np.asarray(w2), np.asarray(b2))
    res = run_bass_kernel_spmd(nc, in_maps, core_ids=list(range(NCORES)))
    return postprocess([res.results[k]['out'] for k in range(NCORES)])
